# revision 17
# baseline (speedup 1.0000x reference)
"""Trainium2 Bass kernel for a dense transformer block (pre-LN attention +
GELU MLP) — fp8e4m3 DoubleRow edition.

Strategy: data-parallel over batch across 8 NeuronCores (2 batches/core, no
collectives).  All matmuls run in fp8e4m3 with MatmulPerfMode.DoubleRow
(2 k-tiles per instruction at 0.5 cycles/row = 4x the fp32r rate).  Accuracy
is held by:
  - weights pre-scaled by 64/128 into fp8's normal range (inverse scale is
    folded into the free scalar slots of psum->SBUF copy ops),
  - residual-split operands: the MLP input and hidden activations are
    represented as main+residual fp8 pairs (a1+a2, h1+h2), and w1/w2 carry a
    matched-scale fp8 residual term (w1b/w2b), so the MLP is computed to
    ~0.3% while still running entirely at DoubleRow rate,
  - softmax without max-subtraction: es = exp(s - 3.2) stored in fp8 (the
    constant bias cancels in the normalization; max score ~8.2 so es < 240).
LayerNorm beta terms are folded into the following layer's biases on the host
(exactly linear), so the on-chip LN applies only the gain.

Engine-overhead notes: psum accumulation groups span disjoint column
sub-chunks of one bank (start zeroes the whole 2KB region), so ACT/DVE
psum->SBUF ops cover 512-col blocks; the 8 per-tile LN transposes land in one
bank (8 x 128 bf16 = 2KB) and are applied with a single DVE op; gelu runs one
ACT pass to bf16 with the fp8 split produced on the (otherwise idle) GpSimd.
Streamed weights are pre-blocked on the host so every DMA descriptor is a
contiguous 4-8KB per-partition run.
"""

import numpy as np
import ml_dtypes

import concourse.bass as bass
import concourse.mybir as mybir
import concourse.tile as tile
from concourse import bacc, bass_utils
from concourse.masks import make_identity

# Problem shape (hardcoded per spec nn_Block_58652073394865)
B, S, D, H, F = 16, 577, 1024, 16, 4096
DH = D // H
NCORES = 8
BL = B // NCORES        # batches per core
P = 128
KK = D // P             # 8 chunks of the model dim
FK = F // P             # 32 chunks of the mlp dim
EPS = 1e-6

SP = 578                # tokens padded with one zero token
TT = [(0, 128), (128, 128), (256, 128), (384, 128), (512, 66)]
# psum groups: [0,512) as two 256-wide DR sub-chunks in one bank, then [512,578)
CH = [(0, 512, (256, 256)), (512, 66, (66,))]
SC = [(0, 512), (512, 66)]               # plain-fp8 score chunks (out <= 512)
VS = 66                 # per-head stride in v (64 v + 1 ones + 1 pad)

WSC = 64.0              # fp8 pre-scale for wq/wk/wv/wo/w1
W2SC = 128.0            # fp8 pre-scale for w2
CTXSC = 16.0            # fp8 pre-scale for ctx
EXPB = 3.2              # softmax exp bias (cancels in normalization)
FC1_TERMS = 3           # 2: a1@w1a + a2@w1a;  3: + a1@w1b
FC2_TERMS = 3           # 2: h1@w2a + h2@w2a;  3: + h1@w2b

F32 = mybir.dt.float32
BF16 = mybir.dt.bfloat16
FP8 = mybir.dt.float8e4
U32 = mybir.dt.uint32
AF = mybir.ActivationFunctionType
OP = mybir.AluOpType
DR = mybir.MatmulPerfMode.DoubleRow

E4NP = ml_dtypes.float8_e4m3
BFNP = ml_dtypes.bfloat16

_NC_CACHE = None
# CoreSim doesn't implement the Gelu LUT; tests may swap this for AF.Tanh
_GELU = AF.Gelu

SHARED_NAMES = ["wq", "wk", "wv", "wo", "w1a", "w2a", "bq", "bk", "bv", "bo",
                "b1", "b2"]
if FC1_TERMS == 3:
    SHARED_NAMES.append("w1b")
if FC2_TERMS == 3:
    SHARED_NAMES.append("w2b")


def _block_layout(w8, nblk, cols):
    """[K, N] fp8 -> [nblk, 128, (K//128) * cols] with per-partition
    contiguous (ko, col) runs, matching SBUF tiles [P, K//128, cols]."""
    Kd, Nd = w8.shape
    ko = Kd // P
    # arr[b, p, k, c] = w8[k*128 + p, b*cols + c]
    a = w8.reshape(ko, P, nblk, cols).transpose(2, 1, 0, 3)
    return np.ascontiguousarray(a.reshape(nblk, P, ko * cols))


def prepare_shared(inputs):
    """Host-side: quantize/scale weights, fold LN betas into biases."""
    f = {n: np.ascontiguousarray(np.asarray(inputs[n], np.float32))
         for n in ("wq", "wk", "wv", "wo", "w1", "w2", "bq", "bk", "bv", "bo",
                   "b1", "b2", "ln1_g", "ln1_b", "ln2_g", "ln2_b")}

    def q8s(w, s):
        return (w * s).astype(E4NP)

    g1 = f["ln1_g"][:, None]
    g2 = f["ln2_g"][:, None]
    out = {
        "wq": _block_layout(q8s(g1 * f["wq"], WSC), 2, 512),
        "wk": _block_layout(q8s(g1 * f["wk"], WSC), 2, 512),
        "wv": _block_layout(q8s(g1 * f["wv"], WSC), 2, 512),
        "wo": _block_layout(q8s(f["wo"], WSC), 2, 512),
        "bq": f["bq"] + f["ln1_b"] @ f["wq"],
        "bk": f["bk"] + f["ln1_b"] @ f["wk"],
        "bv": np.ascontiguousarray((f["bv"] + f["ln1_b"] @ f["wv"]).astype(BFNP)),
        "bo": np.ascontiguousarray(
            (f["bo"] * (WSC * CTXSC)).astype(BFNP)),
        "b1": f["b1"] + f["ln2_b"] @ f["w1"],
        "b2": f["b2"],
    }
    w1s = (g2 * f["w1"]) * WSC
    w1a = w1s.astype(E4NP)
    out["w1a"] = _block_layout(w1a, 8, 512)
    if FC1_TERMS == 3:
        out["w1b"] = _block_layout(
            (w1s - w1a.astype(np.float32)).astype(E4NP), 8, 512)
    w2s = f["w2"] * W2SC
    w2a = w2s.astype(E4NP)
    out["w2a"] = _block_layout(w2a, 8, 128)
    if FC2_TERMS == 3:
        out["w2b"] = _block_layout(
            (w2s - w2a.astype(np.float32)).astype(E4NP), 8, 128)
    return out


def _build():
    nc = bacc.Bacc("TRN2", target_bir_lowering=False, debug=False,
                   num_devices=NCORES)

    x_d = nc.dram_tensor("x", [BL, S, D], BF16, kind="ExternalInput").ap()
    y_d = nc.dram_tensor("y", [BL, S, D], BF16, kind="ExternalOutput").ap()
    wq_d = nc.dram_tensor("wq", [2, P, KK * 512], FP8, kind="ExternalInput").ap()
    wk_d = nc.dram_tensor("wk", [2, P, KK * 512], FP8, kind="ExternalInput").ap()
    wv_d = nc.dram_tensor("wv", [2, P, KK * 512], FP8, kind="ExternalInput").ap()
    wo_d = nc.dram_tensor("wo", [2, P, KK * 512], FP8, kind="ExternalInput").ap()
    w1a_d = nc.dram_tensor("w1a", [8, P, KK * 512], FP8, kind="ExternalInput").ap()
    w2a_d = nc.dram_tensor("w2a", [8, P, FK * 128], FP8, kind="ExternalInput").ap()
    w1b_d = (nc.dram_tensor("w1b", [8, P, KK * 512], FP8,
                            kind="ExternalInput").ap()
             if FC1_TERMS == 3 else None)
    w2b_d = (nc.dram_tensor("w2b", [8, P, FK * 128], FP8,
                            kind="ExternalInput").ap()
             if FC2_TERMS == 3 else None)
    bq_d = nc.dram_tensor("bq", [D], F32, kind="ExternalInput").ap()
    bk_d = nc.dram_tensor("bk", [D], F32, kind="ExternalInput").ap()
    bv_d = nc.dram_tensor("bv", [D], BF16, kind="ExternalInput").ap()
    bo_d = nc.dram_tensor("bo", [D], BF16, kind="ExternalInput").ap()
    b1_d = nc.dram_tensor("b1", [F], F32, kind="ExternalInput").ap()
    b2_d = nc.dram_tensor("b2", [D], F32, kind="ExternalInput").ap()

    with tile.TileContext(nc) as tc:
        with tc.tile_pool(name="const", bufs=1) as cpool, \
             tc.tile_pool(name="resid", bufs=1) as rpool, \
             tc.tile_pool(name="fmbuf", bufs=1) as fmpool, \
             tc.tile_pool(name="lnp", bufs=2) as lnpool, \
             tc.tile_pool(name="ystg", bufs=1) as ypool, \
             tc.tile_pool(name="psA", bufs=4, space="PSUM") as psA:

            # ---- small params, packed ----
            cA = cpool.tile([P, 3 * KK + FK], F32, tag="cA")
            bq_sb = cA[:, 0:KK]
            bk_sb = cA[:, KK:2 * KK]
            b2_sb = cA[:, 2 * KK:3 * KK]
            b1_sb = cA[:, 3 * KK:3 * KK + FK]
            nc.gpsimd.dma_start(bq_sb, bq_d.rearrange("(m p) -> p m", p=P))
            nc.gpsimd.dma_start(bk_sb, bk_d.rearrange("(m p) -> p m", p=P))
            nc.gpsimd.dma_start(b2_sb, b2_d.rearrange("(m p) -> p m", p=P))
            nc.gpsimd.dma_start(b1_sb, b1_d.rearrange("(m p) -> p m", p=P))

            cB = cpool.tile([P, P + 3], F32, tag="cB")
            ident_f = cB[:, 0:P]
            epsap = cB[:, P:P + 1]
            onef = cB[:, P + 1:P + 2]
            nexpb = cB[:, P + 2:P + 3]
            make_identity(nc, ident_f)
            nc.vector.memset(epsap, EPS)
            nc.vector.memset(onef, 1.0)
            nc.vector.memset(nexpb, -EXPB)

            cC = cpool.tile([P, P + 4], BF16, tag="cC")
            ident_bf = cC[:, 0:P]
            nc.vector.tensor_copy(ident_bf, ident_f)
            cD = cpool.tile([1, P + D], BF16, tag="cD")
            ones_bf = cD[:, 0:P]
            bo_sb = cD[:, P:P + D]
            nc.vector.memset(ones_bf, 1.0)
            nc.gpsimd.dma_start(bo_sb, bo_d[None, :])

            onec8 = cpool.tile([P, 4], FP8, tag="onec8")
            nc.vector.tensor_copy(onec8[:, 0:1], onef)

            # bv broadcast to all partitions (bias varies along free dim)
            bvrow = cpool.tile([1, D], BF16, tag="bvrow")
            nc.gpsimd.dma_start(bvrow[:], bv_d[None, :])
            bvb = cpool.tile([P, D], BF16, tag="bvb")
            nc.gpsimd.partition_broadcast(bvb[:], bvrow[:])

            # ---- LayerNorm helpers (token-major stats; gain-only apply) ----
            def ln_new_stats():
                stats = lnpool.tile([P, 25], F32, tag="stats")
                nc.vector.memset(stats[:, 0:5], 0.0)
                nc.vector.memset(stats[:, 5:10], 1.0)
                return stats

            def ln_tile_stats(stats, src, ti, pt):
                # negmu = -mean(x); ssq = sum(x^2)  (var = ssq/D - mu^2)
                negmu = stats[:, 0:5]
                ssq = stats[:, 5:10]
                nc.vector.tensor_reduce(
                    negmu[:pt, ti:ti + 1], src[:pt, ti],
                    mybir.AxisListType.X, OP.add)
                nc.vector.tensor_scalar_mul(
                    negmu[:pt, ti:ti + 1], negmu[:pt, ti:ti + 1], -1.0 / D)
                scr = lnpool.tile([P, D], BF16, tag="xsq", bufs=1)
                nc.vector.tensor_tensor_reduce(
                    scr[:pt], src[:pt, ti], src[:pt, ti], 1.0, 0.0,
                    OP.mult, OP.add, accum_out=ssq[:pt, ti:ti + 1])

            def ln_finalize(stats, lo, hi):
                negmu = stats[:, 0 + lo:0 + hi]
                ssq = stats[:, 5 + lo:5 + hi]
                varr = stats[:, 10 + lo:10 + hi]
                sig = stats[:, 15 + lo:15 + hi]
                rsig = stats[:, 20 + lo:20 + hi]
                nc.vector.tensor_tensor(varr, negmu, negmu, OP.mult)
                nc.vector.scalar_tensor_tensor(
                    varr, ssq, 1.0 / D, varr, OP.mult, OP.subtract)
                nc.scalar.activation(sig, varr, AF.Sqrt, bias=epsap[:])
                nc.vector.reciprocal(rsig, sig)

            def ln_apply_tiles(stats, src, dst1, dst2, tis):
                # normalize (bf16), 8 PE-transposes into ONE psum bank, then
                # plain fp8 copies (LN gains are folded into the weights).
                # dst2 (if not None) receives the fp8 residual split.
                negmu = stats[:, 0:5]
                rsig = stats[:, 20:25]
                for ti in tis:
                    t0, pt = TT[ti]
                    xn = lnpool.tile([P, D], BF16, tag="xn_tm", bufs=2)
                    nc.vector.tensor_scalar(
                        xn[:pt], src[:pt, ti],
                        negmu[:pt, ti:ti + 1], rsig[:pt, ti:ti + 1],
                        OP.add, OP.mult)
                    pst = psA.tile([P, 512], F32, tag="pA", bufs=2, name="pst")
                    pbf = pst[:].bitcast(BF16).rearrange(
                        "p (k c) -> p k c", c=P)
                    for kk in range(KK):
                        nc.tensor.matmul(
                            pbf[:, kk, :pt],
                            xn[:pt, kk * P:(kk + 1) * P],
                            ident_bf[:pt, :pt],
                            is_transpose=True,
                            start=(kk == 0), stop=(kk == KK - 1))
                    nc.scalar.copy(dst1[:, :, t0:t0 + pt], pbf[:, :, :pt])
                    if dst2 is not None:
                        nc.vector.tensor_tensor(
                            dst2[:, :, t0:t0 + pt], pbf[:, :, :pt],
                            dst1[:, :, t0:t0 + pt], OP.subtract)

            def load_x(xb, b):
                nc.vector.memset(xb[64:, 4, :].bitcast(U32), 0)
                for ti, (t0, pt) in enumerate(TT):
                    rp = min(pt, S - t0)
                    nc.sync.dma_start(xb[:rp, ti], x_d[b, t0:t0 + rp, :])

            # ================= per-batch stages =================

            def gen_qkv(apool, xn1_fm, q8, k8, v_sb):
                # Q/K: feature-major out; V: token-major out (+ones col).
                nc.vector.memset(v_sb[:].bitcast(U32), 0)
                v_hc = v_sb[:].rearrange("p t (h c) -> p t h c", c=VS)
                nc.vector.tensor_copy(
                    v_hc[:, 0:4, :, 64:65],
                    onec8[:, 0:1, None, None].to_broadcast((P, 4, H, 1)))
                nc.vector.tensor_copy(
                    v_hc[:65, 4:5, :, 64:65],
                    onec8[:65, 0:1, None, None].to_broadcast((65, 1, H, 1)))

                for w_d, bias_sb, dst, use_act in ((wq_d, bq_sb, q8, True),
                                                   (wk_d, bk_sb, k8, False)):
                    for blk in range(2):
                        yield
                        wt = apool.tile([P, KK, 512], FP8, tag="wqkv",
                                        bufs=4, name="wqkv")
                        nc.sync.dma_start(
                            wt[:], w_d[blk].rearrange("p (k c) -> p k c",
                                                      c=512))
                        for mi in range(4):
                            m = blk * 4 + mi
                            for (c0, cw, subs) in CH:
                                ps = psA.tile([P, 512], F32, tag="pA", bufs=2, name="ps")
                                nsub = len(subs)
                                for si in range(nsub):
                                    s0 = c0 + si * 256
                                    sn = subs[si]
                                    for kp in range(4):
                                        nc.tensor.matmul(
                                            ps[:, si * 256:si * 256 + sn],
                                            wt[:, 2 * kp:2 * kp + 2,
                                               mi * P:(mi + 1) * P],
                                            xn1_fm[:, 2 * kp:2 * kp + 2,
                                                   s0:s0 + sn],
                                            start=(si == 0 and kp == 0),
                                            stop=(si == nsub - 1 and kp == 3),
                                            perf_mode=DR)
                                if use_act:
                                    nc.scalar.activation(
                                        dst[:, m, c0:c0 + cw], ps[:, :cw],
                                        AF.Identity,
                                        bias=bias_sb[:, m:m + 1],
                                        scale=1.0 / WSC)
                                else:
                                    nc.vector.tensor_scalar(
                                        dst[:, m, c0:c0 + cw], ps[:, :cw],
                                        1.0 / WSC, bias_sb[:, m:m + 1],
                                        OP.mult, OP.add)

                for blk in range(2):
                    yield
                    wt = apool.tile([P, KK, 512], FP8, tag="wqkv",
                                    bufs=4, name="wqkv")
                    nc.sync.dma_start(
                        wt[:], wv_d[blk].rearrange("p (k c) -> p k c", c=512))
                    for ti, (t0, pt) in enumerate(TT):
                        rp = min(pt, S - t0)
                        ps = psA.tile([P, 512], F32, tag="pA", bufs=2, name="ps")
                        for cc in range(2):
                            for kp in range(4):
                                nc.tensor.matmul(
                                    ps[:pt, cc * 256:(cc + 1) * 256],
                                    xn1_fm[:, 2 * kp:2 * kp + 2, t0:t0 + pt],
                                    wt[:, 2 * kp:2 * kp + 2,
                                       cc * 256:(cc + 1) * 256],
                                    start=(cc == 0 and kp == 0),
                                    stop=(cc == 1 and kp == 3),
                                    perf_mode=DR)
                        c0 = blk * 512
                        nc.vector.scalar_tensor_tensor(
                            v_hc[:rp, ti, 8 * blk:8 * (blk + 1), 0:64],
                            ps[:rp, :512].rearrange("p (h c) -> p h c", c=64),
                            1.0 / WSC,
                            bvb[:rp, c0:c0 + 512].rearrange(
                                "p (h c) -> p h c", c=64),
                            OP.mult, OP.add)

            def emit_scores(h, q8, k8, es):
                # es[:, kt, q] = exp(q.k/8 - EXPB) in fp8
                hrow = (h % 2) * 64
                kkh = h // 2
                for (c0, cn) in SC:
                    for pair in ((0, 1), (2, 3), (4,)):
                        pg = psA.tile([P, 2, 512], F32, tag="pS", bufs=2,
                                      name="pg")
                        for j, kt in enumerate(pair):
                            t0, ptk = TT[kt]
                            nc.tensor.matmul(
                                pg[:ptk, j, :cn],
                                k8[hrow:hrow + 64, kkh, t0:t0 + ptk],
                                q8[hrow:hrow + 64, kkh, c0:c0 + cn],
                                start=True, stop=True)
                        npair = len(pair)
                        prow = TT[pair[0]][1]
                        nc.scalar.activation(
                            es[:prow, pair[0]:pair[0] + npair, c0:c0 + cn],
                            pg[:prow, :npair, :cn],
                            AF.Exp, scale=1.0 / np.sqrt(DH),
                            bias=nexpb[:prow])

            def emit_pv(h, es, v_sb, ctx_fm):
                hrow = (h % 2) * 64
                kkh = h // 2
                for (c0, cw, subs) in CH:
                    pc = psA.tile([P, 512], F32, tag="pV", bufs=2, name="pc")
                    nsub = len(subs)
                    for si in range(nsub):
                        s0 = c0 + si * 256
                        sn = subs[si]
                        for kp in range(2):
                            nc.tensor.matmul(
                                pc[:VS, si * 256:si * 256 + sn],
                                v_sb[:, 2 * kp:2 * kp + 2,
                                     h * VS:(h + 1) * VS],
                                es[:, 2 * kp:2 * kp + 2, s0:s0 + sn],
                                start=(si == 0 and kp == 0), stop=False,
                                perf_mode=DR)
                        nc.tensor.matmul(
                            pc[:VS, si * 256:si * 256 + sn],
                            v_sb[:66, 4, h * VS:(h + 1) * VS],
                            es[:66, 4, s0:s0 + sn],
                            start=False, stop=(si == nsub - 1))
                    rz = lnpool.tile([65, 512], BF16, tag="rz", bufs=2)
                    rc = rz[64:65]
                    rb = rz[0:64]
                    with nc.allow_low_precision(reason="softmax 1/Z bf16"):
                        nc.vector.reciprocal(rc[:, :cw], pc[64:65, :cw])
                    nc.gpsimd.partition_broadcast(rb[:, :cw], rc[:, :cw])
                    nc.vector.scalar_tensor_tensor(
                        ctx_fm[hrow:hrow + 64, kkh, c0:c0 + cw],
                        pc[0:64, :cw], CTXSC, rb[:, :cw],
                        OP.mult, OP.mult)

            def prefetch_wo():
                tiles = []
                for blk in range(2):
                    wt = apool_ref[0].tile([P, KK, 512], FP8, tag="wqkv",
                                           bufs=4, name="wqkv")
                    nc.sync.dma_start(
                        wt[:], wo_d[blk].rearrange("p (k c) -> p k c", c=512))
                    tiles.append(wt)
                return tiles

            def gen_oproj(wo_tiles, ctx_fm, xb, x2, stats2):
                # out token-major: x2 = attn/(WSC*CTXSC) + x ; LN2 stats after
                for blk in range(2):
                    yield
                    wt = wo_tiles[blk]
                    c0 = blk * 512
                    for ti in (4, 0, 1, 2, 3):
                        t0, pt = TT[ti]
                        ps = psA.tile([P, 512], F32, tag="pA", bufs=2, name="ps")
                        for cc in range(2):
                            for kp in range(4):
                                nc.tensor.matmul(
                                    ps[:pt, cc * 256:(cc + 1) * 256],
                                    ctx_fm[:, 2 * kp:2 * kp + 2, t0:t0 + pt],
                                    wt[:, 2 * kp:2 * kp + 2,
                                       cc * 256:(cc + 1) * 256],
                                    start=(cc == 0 and kp == 0), stop=False,
                                    perf_mode=DR)
                        nc.tensor.matmul(
                            ps[:pt, :512], ones_bf[:1, :pt],
                            bo_sb[:1, c0:c0 + 512], start=False, stop=True)
                        nc.vector.scalar_tensor_tensor(
                            x2[:pt, ti, c0:c0 + 512], ps[:pt, :512],
                            1.0 / (WSC * CTXSC), xb[:pt, ti, c0:c0 + 512],
                            OP.mult, OP.add)
                yield
                for ti, (t0, pt) in enumerate(TT):
                    ln_tile_stats(stats2, x2, ti, pt)

            def gen_fc1(a1, a2, h1, h2, mpool, mwpool):
                for blk in range(8):
                    yield
                    wa = mwpool.tile([P, KK, 512], FP8, tag="w1a", bufs=2)
                    nc.sync.dma_start(
                        wa[:], w1a_d[blk].rearrange("p (k c) -> p k c",
                                                    c=512))
                    if FC1_TERMS == 3:
                        wb = mwpool.tile([P, KK, 512], FP8, tag="w1b",
                                         bufs=2)
                        nc.sync.dma_start(
                            wb[:], w1b_d[blk].rearrange("p (k c) -> p k c",
                                                        c=512))
                    for mi in range(4):
                        m = blk * 4 + mi
                        mc = slice(mi * P, (mi + 1) * P)
                        hb = mpool.tile([P, SP], BF16, tag="hb", bufs=2)
                        for (c0, cw, subs) in CH:
                            ps = psA.tile([P, 512], F32, tag="pA", bufs=2, name="ps")
                            terms = [(a1, wa), (a2, wa)]
                            if FC1_TERMS == 3:
                                terms.append((a1, wb))
                            nterm = len(terms)
                            nsub = len(subs)
                            for si in range(nsub):
                                s0 = c0 + si * 256
                                sn = subs[si]
                                for tix, (asrc, wsrc) in enumerate(terms):
                                    for kp in range(4):
                                        nc.tensor.matmul(
                                            ps[:, si * 256:si * 256 + sn],
                                            wsrc[:, 2 * kp:2 * kp + 2, mc],
                                            asrc[:, 2 * kp:2 * kp + 2,
                                                 s0:s0 + sn],
                                            start=(si == 0 and tix == 0
                                                   and kp == 0),
                                            stop=(si == nsub - 1 and
                                                  tix == nterm - 1 and
                                                  kp == 3),
                                            perf_mode=DR)
                            nc.scalar.activation(
                                hb[:, c0:c0 + cw], ps[:, :cw], _GELU,
                                bias=b1_sb[:, m:m + 1], scale=1.0 / WSC)
                        nc.gpsimd.tensor_copy(h1[:, m, :], hb[:, :])
                        nc.gpsimd.tensor_tensor(
                            h2[:, m, :], hb[:, :], h1[:, m, :], OP.subtract)

            def gen_fc2(h1, h2, x2, y_sb, mpool, mwpool):
                for blk in range(8):
                    yield
                    wa = mwpool.tile([P, FK, 128], FP8, tag="w2a", bufs=2)
                    nc.sync.dma_start(
                        wa[:], w2a_d[blk].rearrange("p (k c) -> p k c",
                                                    c=128))
                    if FC2_TERMS == 3:
                        wb = mwpool.tile([P, FK, 128], FP8, tag="w2b",
                                         bufs=2)
                        nc.sync.dma_start(
                            wb[:], w2b_d[blk].rearrange("p (k c) -> p k c",
                                                        c=128))
                    for mi in range(1):
                        m = blk
                        mc = slice(0, P)
                        mlp_fm = mpool.tile([P, SP], BF16, tag="mlp_fm",
                                            bufs=2)
                        for (c0, cw, subs) in CH:
                            ps = psA.tile([P, 512], F32, tag="pA", bufs=2, name="ps")
                            terms = [(h1, wa), (h2, wa)]
                            if FC2_TERMS == 3:
                                terms.append((h1, wb))
                            nterm = len(terms)
                            nsub = len(subs)
                            for si in range(nsub):
                                s0 = c0 + si * 256
                                sn = subs[si]
                                for tix, (hsrc, wsrc) in enumerate(terms):
                                    for kp in range(FK // 2):
                                        nc.tensor.matmul(
                                            ps[:, si * 256:si * 256 + sn],
                                            wsrc[:, 2 * kp:2 * kp + 2, mc],
                                            hsrc[:, 2 * kp:2 * kp + 2,
                                                 s0:s0 + sn],
                                            start=(si == 0 and tix == 0
                                                   and kp == 0),
                                            stop=(si == nsub - 1 and
                                                  tix == nterm - 1 and
                                                  kp == FK // 2 - 1),
                                            perf_mode=DR)
                            nc.vector.tensor_scalar(
                                mlp_fm[:, c0:c0 + cw], ps[:, :cw],
                                1.0 / W2SC, b2_sb[:, m:m + 1],
                                OP.mult, OP.add)
                        # 5 transposes into one bank; single residual op.
                        # (tile0's start zeroes the whole region, so the
                        # garbage rows of tile4 read as zeros.)
                        pst = psA.tile([P, 512], F32, tag="pA", bufs=2, name="pst")
                        pbf = pst[:].bitcast(BF16).rearrange(
                            "p (t c) -> p t c", c=P)
                        for ti, (t0, pt) in enumerate(TT):
                            nc.tensor.matmul(
                                pbf[:pt, ti, :P], mlp_fm[:, t0:t0 + pt],
                                ident_bf[:], is_transpose=True,
                                start=(ti == 0), stop=(ti == 4))
                        nc.vector.scalar_tensor_tensor(
                            y_sb[:, :, m * P:(m + 1) * P],
                            pbf[:, 0:5, :], 0.0,
                            x2[:, :, m * P:(m + 1) * P],
                            OP.add, OP.add)

            # ======== software-pipelined batch schedule ========
            # A(b)=LN1+QKV, B(b)=heads, C(b)=Oproj+LN2, MLP(b)=fc1+fc2.
            # Emission (=per-engine execution) order:
            #   A0, B0(x)A1, B1(x)[C0,MLP0...], C1(x)[MLP0...], MLP1
            # so the ACT-bound attention windows are covered by the other
            # batch's PE-bound stages.
            apool_ref = [None]
            with tc.tile_pool(name="attn", bufs=1) as apool, \
                 tc.tile_pool(name="mlp", bufs=1) as mpool, \
                 tc.tile_pool(name="wmlp", bufs=1) as mwpool:
                apool_ref[0] = apool

                T = [dict() for _ in range(BL)]

                def gen_A(b):
                    t = T[b]
                    xb = rpool.tile([P, 5, D], BF16, tag="xb", bufs=2,
                                    name="xb")
                    t["xb"] = xb
                    load_x(xb, b)
                    stats1 = ln_new_stats()
                    for ti, (t0, pt) in enumerate(TT):
                        ln_tile_stats(stats1, xb, ti, pt)
                    yield
                    xn1 = fmpool.tile([P, KK, SP], FP8, tag="xn1_fm",
                                      name="xn1")
                    ln_finalize(stats1, 0, 2)
                    ln_apply_tiles(stats1, xb, xn1, None, (0, 1))
                    yield
                    ln_finalize(stats1, 2, 5)
                    ln_apply_tiles(stats1, xb, xn1, None, (2, 3, 4))
                    q8 = apool.tile([P, KK, SP], FP8, tag="q8", bufs=2,
                                    name="q8")
                    k8 = apool.tile([P, KK, SP], FP8, tag="k8", bufs=2,
                                    name="k8")
                    v_sb = apool.tile([P, 5, H * VS], FP8, tag="v", bufs=2,
                                      name="v_sb")
                    t["q8"], t["k8"], t["v"] = q8, k8, v_sb
                    yield from gen_qkv(apool, xn1, q8, k8, v_sb)

                def gen_B(b):
                    t = T[b]
                    ctx_fm = apool.tile([P, KK, SP], FP8, tag="ctx",
                                        name="ctx_fm")
                    t["ctx"] = ctx_fm
                    es_cur = apool.tile([P, 5, SP], FP8, tag="es0",
                                        name="es_cur")
                    emit_scores(0, t["q8"], t["k8"], es_cur)
                    for h in range(H):
                        if h + 1 < H:
                            es_nxt = apool.tile([P, 5, SP], FP8,
                                                tag=f"es{(h + 1) % 2}",
                                                name="es_nxt")
                            emit_scores(h + 1, t["q8"], t["k8"], es_nxt)
                        emit_pv(h, es_cur, t["v"], ctx_fm)
                        if h + 1 < H:
                            es_cur = es_nxt
                        yield

                def gen_C(b, wo_tiles):
                    t = T[b]
                    x2 = rpool.tile([P, 5, D], BF16, tag="x2", bufs=2,
                                    name="x2")
                    t["x2"] = x2
                    stats2 = ln_new_stats()
                    yield from gen_oproj(wo_tiles, t["ctx"], t["xb"], x2,
                                         stats2)
                    a1 = fmpool.tile([P, KK, SP], FP8, tag="a1", name="a1")
                    a2 = fmpool.tile([P, KK, SP], FP8, tag="a2", name="a2")
                    t["a1"], t["a2"] = a1, a2
                    ln_finalize(stats2, 0, 5)
                    for ti in range(5):
                        ln_apply_tiles(stats2, x2, a1, a2, (ti,))
                        yield

                def gen_mlp(b):
                    t = T[b]
                    h1 = mpool.tile([P, FK, SP], FP8, tag="h1", name="h1")
                    h2 = mpool.tile([P, FK, SP], FP8, tag="h2", name="h2")
                    yield from gen_fc1(t["a1"], t["a2"], h1, h2,
                                       mpool, mwpool)
                    y_sb = ypool.tile([P, 5, D], BF16, tag="y_sb",
                                      name="y_sb")
                    t["y"] = y_sb
                    yield from gen_fc2(h1, h2, t["x2"], y_sb, mpool, mwpool)

                def store_y(b):
                    for ti, (t0, pt) in enumerate(TT):
                        rp = min(pt, S - t0)
                        nc.sync.dma_start(y_d[b, t0:t0 + rp, :],
                                          T[b]["y"][:rp, ti])

                def run_all(g):
                    for _ in g:
                        pass

                def interleave(main, side, ratio):
                    acc = 0.0
                    for _ in main:
                        acc += ratio
                        while acc >= 1.0:
                            next(side, None)
                            acc -= 1.0

                from itertools import chain as _chain

                run_all(gen_A(0))
                wo0 = prefetch_wo()
                interleave(gen_B(0), gen_A(1), 10.0 / H)
                wo1 = prefetch_wo()
                side = _chain(gen_C(0, wo0), gen_mlp(0))
                interleave(gen_B(1), side, 17.0 / H)
                interleave(gen_C(1, wo1), side, 2.0)
                run_all(side)
                store_y(0)
                run_all(gen_mlp(1))
                store_y(1)

    nc.compile()
    return nc


def _get_nc():
    global _NC_CACHE
    if _NC_CACHE is None:
        _NC_CACHE = _build()
    return _NC_CACHE


def kernel(**inputs):
    nc = _get_nc()
    shared = prepare_shared(inputs)
    x = np.asarray(inputs["x"], np.float32).astype(BFNP)
    in_maps = []
    for i in range(NCORES):
        m = dict(shared)
        m["x"] = np.ascontiguousarray(x[i * BL:(i + 1) * BL])
        in_maps.append(m)
    res = bass_utils.run_bass_kernel_spmd(nc, in_maps,
                                          core_ids=list(range(NCORES)))
    y = np.concatenate([np.asarray(res.results[i]["y"])
                        for i in range(NCORES)], axis=0)
    return y.astype(np.float32)
